# revision 4
# baseline (speedup 1.0000x reference)
"""Embedding lookup (gather) kernel for Trainium2, 8 NeuronCores.

Problem: out[i] = table[value_tensors[i]] for 212992 indices into a
[1M, 128] f32 table, reshaped to [8192, 26, 128]. (row_offsets is
arange, so the CSR segment-sum is the identity; a host-side fallback
handles the general case.)

Sharding: model-parallel by table row (range partition). The table is
split into 32 range bins of 31250 rows; core c owns bins 4c..4c+3.
The host routes each lookup index to its owning bin, each core gathers
its rows on-device with the SWDGE dma_gather instruction, and the host
scatters the gathered rows back to the original positions (the
"all-to-all" step of HugeCTR's localized embedding, done at unshard
time).

Perf strategy (measured: f32 baseline 126us; fp16 halves both HBM
directions -> 88us; the gather is then DESCRIPTOR-rate bound, not
byte bound: ~100 descs/us/queue at either 256B or 512B descs,
i.e. ~30ns/desc/engine ~= HBM read latency / ~7 outstanding):
  - fp16 table + fp16 output, host upconverts to f32. Max rel error
    2^-11 vs the 2e-2 gate.
  - dedupe lookup indices on host (~10% repeats); expand at unshard.
  - engine-contiguous idx layout: descs are issued ring r = partition
    r; giving ring r a CONTIGUOUS run of the sorted uniques makes each
    SDMA engine's HBM walk monotonic with ~1.2KB strides (page/bank
    locality) instead of stride-16 jumps.
  - 8 semaphores total (was 42): the finalize barrier retires a
    serial per-sem wait train on every engine (~0.1us each).
  - warmup dma_gather per queue overlapped with the idx load (first
    SWDGE gather on a cold queue measured ~9us extra).
  - per-bin idx loads so bin 0's gathers start before bins 1-3 land.

dma_gather layout (probed on HW): indices are int16, wrapped over 16
partitions (ordinal i reads idx[i % 16, i // 16]) and replicated to all
8 Q7-core partition groups; gathered ordinal i lands at
dst[i % 128, i // 128].
"""

import math

import numpy as np

VOCAB = 1_000_000
BATCH = 8192
SLOTS = 26
VEC = 128
NCORES = 8
NSUB = 4  # sub-shards (bins) per core; int16 gather idx needs rows <= 32767
RSUB = VOCAB // (NCORES * NSUB)  # 31250 rows per bin
SHARD = RSUB * NSUB  # 125000 rows per core
P = 128
# Idxs per dma_gather: 896 -> 56 data descs + 1 sem desc per engine ring,
# safely under the 64-descriptor packet ceiling (1024 -> 65 descs, which
# is at/over the limit and produced rare device lockups).
CH = 896

LAST_RUN = None  # BassKernelResults of the most recent device run (for test.py)


def _chunks_of(N: int):
    out = []
    o = 0
    while o < N:
        out.append((o, min(CH, N - o)))
        o += CH
    return out


def _build_program(N: int):
    """One SPMD program for all 8 cores. N = padded lookups per bin
    (multiple of 128; identical across cores/bins so num_idxs is a
    compile-time constant).

    Per core:
      shard [SHARD, VEC] fp16     - this core's 4 bins, concatenated
      idx   [P, NSUB*S] int16     - per-ring-contiguous local indices
      out   [P, NSUB*C*VEC] fp16  - gathered rows, C = N//128
    """
    import concourse.bacc as bacc
    from concourse import mybir
    from concourse.library_config import mlp

    S = N // 16
    C = N // 128
    chunks = _chunks_of(N)
    nch = len(chunks)
    half = max(1, nch // 2)

    nc = bacc.Bacc("TRN2", num_swdge_queues=4)
    shard = nc.declare_dram_parameter(
        "shard", [SHARD, VEC], mybir.dt.float16, isOutput=False
    )
    idx = nc.declare_dram_parameter(
        "idx", [P, NSUB * S], mybir.dt.int16, isOutput=False
    )
    out = nc.declare_dram_parameter(
        "out", [P, NSUB * C * VEC], mybir.dt.float16, isOutput=True
    )

    sem_idx = nc.alloc_semaphore("sem_idx")
    sem_warm = nc.alloc_semaphore("sem_warm")
    # One gather sem per whole-bin write group; the last bin is split at
    # the chunk midpoint so the post-gather write tail is half a bin.
    sem_g = [nc.alloc_semaphore(f"sem_g{s}") for s in range(NSUB + 1)]
    sem_out = nc.alloc_semaphore()

    idx_sb = nc.alloc_sbuf_tensor("idx_sb", [P, NSUB * S], mybir.dt.int16).ap()
    warm_idx = nc.alloc_sbuf_tensor("warm_idx", [P, 8], mybir.dt.int16).ap()
    warm_out = nc.alloc_sbuf_tensor("warm_out", [P, 1, VEC], mybir.dt.float16).ap()
    g_bufs = [
        nc.alloc_sbuf_tensor(f"g{s}", [P, C, VEC], mybir.dt.float16).ap()
        for s in range(NSUB)
    ]

    nc.gpsimd.load_library(mlp)
    # Per-bin idx loads on one HWDGE ring: FIFO completion order means
    # sem_idx >= 16*(s+1) <=> bins 0..s have landed.
    for s in range(NSUB):
        nc.sync.dma_start(
            out=idx_sb[:, s * S : (s + 1) * S], in_=idx[:, s * S : (s + 1) * S]
        ).then_inc(sem_idx, 16)

    # Hoist num_idxs registers: one MOVE per distinct chunk size instead of
    # one per gather (each MOVE costs ~420ns of Pool sequencer time).
    sizes = sorted({sz for _, sz in chunks} | {128})
    size_regs = {sz: nc.gpsimd.to_reg(sz) for sz in sizes}

    # Warmup: one tiny gather per queue (row 0 x128) while the idx DMA is
    # still in flight. The first SWDGE gather on a cold queue measured ~9us
    # (ring/ucode init); this absorbs it off the critical path.
    nc.gpsimd.memset(warm_idx, 0)
    for q in range(4):
        nc.gpsimd.dma_gather(
            warm_out[:, :, :],
            shard[0:RSUB, :],
            warm_idx[:, :],
            128,
            size_regs[128],
            VEC,
            queue_num=q,
        ).then_inc(sem_warm, 16)

    # Bin-major issue, queue = position % 4: spreads each bin across all 4
    # queues and staggers write-group completions.
    for s in range(NSUB):
        nc.gpsimd.wait_ge(sem_idx, 16 * (s + 1))
        for j, (o, sz) in enumerate(chunks):
            sem = sem_g[s] if (s < NSUB - 1 or j < half) else sem_g[NSUB]
            nc.gpsimd.dma_gather(
                g_bufs[s][:, o // 128 : (o + sz) // 128, :],
                shard[s * RSUB : (s + 1) * RSUB, :],
                idx_sb[:, s * S + o // 16 : s * S + (o + sz) // 16],
                sz,
                size_regs[sz],
                VEC,
                queue_num=(s * nch + j) % 4,
            ).then_inc(sem, 16)

    # Writeouts (~12.5KB per partition-descriptor for near-peak HWDGE
    # rate), alternating between the two HWDGE rings (Sync ->
    # qSPDynamicHW, Scalar -> qActDynamicHW) so writes overlap gathers.
    groups = []  # (s, first_chunk_j, last_chunk_j, sem, need)
    for s in range(NSUB - 1):
        groups.append((s, 0, nch - 1, sem_g[s], 16 * nch))
    groups.append((NSUB - 1, 0, half - 1, sem_g[NSUB - 1], 16 * half))
    groups.append((NSUB - 1, half, nch - 1, sem_g[NSUB], 16 * (nch - half)))
    n_writes = 0
    for gi, (s, j0, j1, sem, need) in enumerate(groups):
        eng = nc.sync if gi % 2 == 0 else nc.scalar
        eng.wait_ge(sem, need)
        o0 = chunks[j0][0]
        o1 = chunks[j1][0] + chunks[j1][1]
        eng.dma_start(
            out=out[:, (s * C + o0 // 128) * VEC : (s * C + o1 // 128) * VEC],
            in_=g_bufs[s][:, o0 // 128 : o1 // 128, :].rearrange("p c v -> p (c v)"),
        ).then_inc(sem_out, 16)
        n_writes += 1
    nc.sync.wait_ge(sem_out, 16 * n_writes)
    nc.sync.wait_ge(sem_warm, 16 * 4)
    nc.finalize()
    return nc


def _perm_of(N: int) -> np.ndarray:
    """perm[t] = position in the bin's sorted-local array whose row the
    device writes at output ordinal t.

    Within each chunk of sz idxs, ring r (= idx partition r, SDMA engine
    r) reads slots [r*m, (r+1)*m) of the chunk's contiguous sorted run
    (m = sz//16): ordinal j maps to position (j%16)*m + j//16.
    """
    perm = np.empty(N, np.int64)
    for o, sz in _chunks_of(N):
        j = np.arange(sz)
        m = sz // 16
        perm[o + j] = o + (j % 16) * m + j // 16
    return perm


def _gather_on_device(table_f16: np.ndarray, uniq: np.ndarray) -> np.ndarray:
    """emb[i] = table[uniq[i]] (fp16) computed on 8 NeuronCores.

    uniq must be sorted ascending (np.unique output): bins are contiguous
    slices, and each SDMA engine's 256B random reads walk HBM
    monotonically (page locality).
    """
    global LAST_RUN
    from concourse.bass_utils import run_bass_kernel_spmd

    total = uniq.shape[0]
    nbins = NCORES * NSUB
    bin_id = (uniq // RSUB).astype(np.int32)
    local = (uniq - bin_id.astype(np.int64) * RSUB).astype(np.int16)

    counts = np.bincount(bin_id, minlength=nbins)
    assert counts.sum() == total
    bin_start = np.concatenate(([0], np.cumsum(counts)))
    N = max(P, ((int(counts.max()) + P - 1) // P) * P)
    S = N // 16
    C = N // 128
    perm = _perm_of(N)

    in_maps = []
    for c in range(NCORES):
        idx_cols = []
        for s in range(NSUB):
            b = c * NSUB + s
            # Pad with index 0 (a valid row): num_idxs_reg must equal the
            # count of non-negative indices, and it is a compile-time
            # constant shared by all cores.
            li = np.zeros(N, np.int16)
            li[: counts[b]] = local[bin_start[b] : bin_start[b + 1]]
            # Ring-contiguous layout: ordinal j of chunk o reads
            # idx[j%16, o//16 + j//16]; setting idx = wrap(li[perm]) with
            # the standard wrap (element t at [t%16, t//16]) gives ring r
            # the contiguous sorted run [o + r*m, o + (r+1)*m).
            wrapped = np.ascontiguousarray(li[perm].reshape(S, 16).T)
            idx_cols.append(np.tile(wrapped, (8, 1)))
        in_maps.append(
            {
                "shard": np.ascontiguousarray(
                    table_f16[c * SHARD : (c + 1) * SHARD]
                ),
                "idx": np.ascontiguousarray(np.concatenate(idx_cols, axis=1)),
            }
        )

    nc = _build_program(N)
    LAST_RUN = run_bass_kernel_spmd(nc, in_maps, list(range(NCORES)))
    res = LAST_RUN.results

    emb = np.empty((total, VEC), np.float16)
    for c in range(NCORES):
        o = np.asarray(res[c]["out"]).reshape(P, NSUB, C, VEC)
        for s in range(NSUB):
            b = c * NSUB + s
            rows = o[:, s].transpose(1, 0, 2).reshape(N, VEC)
            valid = perm < counts[b]
            emb[bin_start[b] + perm[valid]] = rows[valid]
    return emb


def kernel(table, row_offsets, value_tensors, nnz_array=None, output_shape=None):
    table = np.asarray(table, dtype=np.float32)
    assert table.shape == (VOCAB, VEC)
    v = np.asarray(value_tensors).astype(np.int64).ravel()
    total = v.shape[0]

    table_f16 = table.astype(np.float16)
    uniq, inverse = np.unique(v, return_inverse=True)
    emb_u = _gather_on_device(table_f16, uniq)
    emb = emb_u[inverse].astype(np.float32)

    n_rows = BATCH * SLOTS
    ro = np.asarray(row_offsets).astype(np.int64).ravel()
    if total == n_rows and np.array_equal(ro, np.arange(total + 1)):
        return emb.reshape(BATCH, SLOTS, VEC)
    # General CSR fallback (never hit with the reference's arange offsets):
    # sum-combine values per segment on the host.
    seg = np.searchsorted(ro, np.arange(total), side="right") - 1
    combined = np.zeros((n_rows, VEC), np.float32)
    np.add.at(combined, seg, emb)
    return combined.reshape(BATCH, SLOTS, VEC)


# revision 5
# speedup vs baseline: 1.0762x; 1.0762x over previous
"""Embedding lookup (gather) kernel for Trainium2, 8 NeuronCores.

Problem: out[i] = table[value_tensors[i]] for 212992 indices into a
[1M, 128] f32 table, reshaped to [8192, 26, 128]. (row_offsets is
arange, so the CSR segment-sum is the identity; a host-side fallback
handles the general case.)

Sharding: model-parallel by table row (range partition). The table is
split into 32 range bins of 31250 rows; core c owns bins 4c..4c+3.
The host routes each lookup index to its owning bin, each core gathers
its rows on-device with the SWDGE dma_gather instruction, and the host
scatters the gathered rows back to the original positions (the
"all-to-all" step of HugeCTR's localized embedding, done at unshard
time).

Perf strategy (measured: f32 one-desc-per-row baseline 126us; fp16
halves both HBM directions -> 88us; the gather is then DESCRIPTOR-rate
bound, not byte bound: ~100 descs/us/queue at either 256B or 512B
descs; an engine-contiguous idx permutation measured neutral, so the
rate is a fixed per-descriptor cost, not HBM page locality):
  - fp16 table + fp16 output, host upconverts to f32. Max rel error
    2^-11 vs the 2e-2 gate.
  - dedupe lookup indices on host (~10% repeats); expand at unshard.
  - run coalescing: consecutive unique rows are gathered as PAIRS
    (elem_size=256 over a [RSUB/2, 256] view; separate even/odd-start
    phase views avoid overlapping APs). ~16% fewer descriptors.
  - runtime num_idxs via reg_load from a per-core count table +
    negative idx padding: pad slots generate no descriptors, so the
    per-bin/class padding (compile-time shapes shared across cores) is
    free. Every chunk keeps >= 16 non-negative idxs so all 16 engine
    rings still fire their completion-semaphore descriptor.
  - warmup dma_gather per queue overlapped with the idx load (first
    SWDGE gather on a cold queue measured ~9us of init).
  - per-bin idx loads so bin 0's gathers start before bins 1-3 land.
  - few semaphores / whole-bin writes on the two HWDGE rings,
    alternating Sync/Scalar; the last bin's tiny pair region is the
    only post-gather write tail.

dma_gather layout (probed on HW, incl. runtime-reg + negative-pad +
pair-view semantics via probe_reg.py): indices are int16, wrapped over
16 partitions (ordinal i reads idx[i % 16, i // 16]) and replicated to
all 8 Q7-core partition groups; gathered ordinal i lands at
dst[i % 128, i // 128]; negative idxs at the end are skipped entirely.
"""

import math

import numpy as np

VOCAB = 1_000_000
BATCH = 8192
SLOTS = 26
VEC = 128
NCORES = 8
NSUB = 4  # sub-shards (bins) per core; int16 gather idx needs rows <= 32767
RSUB = VOCAB // (NCORES * NSUB)  # 31250 rows per bin
SHARD = RSUB * NSUB  # 125000 rows per core
P = 128
# Idxs per dma_gather: 896 -> 56 data descs + 1 sem desc per engine ring,
# safely under the 64-descriptor packet ceiling.
CH = 896
NCLS = 3  # singles, even-start pairs, odd-start pairs
ELEM = [VEC, 2 * VEC, 2 * VEC]

LAST_RUN = None  # BassKernelResults of the most recent device run (for test.py)


def _chunks_of(N: int):
    out = []
    o = 0
    while o < N:
        out.append((o, min(CH, N - o)))
        o += CH
    return out


def _decompose(rows: np.ndarray):
    """Split sorted unique local rows into greedy pairs + singles.

    Returns (s_pos, s_row), (pe_pos, pe_idx), (po_pos, po_idx): positions
    are indices into `rows` (== bin-relative unique positions); pair idxs
    are start_row//2 for the even/odd phase pair views.
    """
    n = len(rows)
    if n == 0:
        e = np.empty(0, np.int64)
        return (e, e), (e, e), (e, e)
    isnew = np.ones(n, bool)
    isnew[1:] = np.diff(rows) != 1
    runstart = np.flatnonzero(isnew)
    runid = np.cumsum(isnew) - 1
    pos_in_run = np.arange(n) - runstart[runid]
    runlen = np.bincount(runid)
    L = runlen[runid]
    pairstart = (pos_in_run % 2 == 0) & (pos_in_run + 1 < L)
    single = (pos_in_run % 2 == 0) & (pos_in_run + 1 >= L)
    s_pos = np.flatnonzero(single)
    p_pos = np.flatnonzero(pairstart)
    p_row = rows[p_pos]
    even = (p_row % 2) == 0
    return (
        (s_pos, rows[s_pos]),
        (p_pos[even], p_row[even] // 2),
        (p_pos[~even], (p_row[~even] - 1) // 2),
    )


def _build_program(NCL: list, ncalls_bin: int):
    """One SPMD program for all 8 cores. NCL[c] = padded idx slots for
    class c per bin (multiples of 128, identical across cores/bins).

    Per core:
      shard [SHARD, VEC] fp16   - this core's 4 bins, concatenated
      idx   [P, ICOLS] i16      - [8 warm cols][bin0 c0|c1|c2][bin1 ...]
      cnt   [1, NCALL] i32      - per-gather-call runtime num_idxs
      out   [P, NSUB*W] fp16    - W = per-bin output cols
    """
    import concourse.bacc as bacc
    from concourse import mybir
    from concourse.library_config import mlp

    chunks = [_chunks_of(NCL[c]) for c in range(NCLS)]
    icols_bin = sum(NCL) // 16
    W = NCL[0] + (NCL[1] // 128) * 256 + (NCL[2] // 128) * 256
    roff = [0, NCL[0], NCL[0] + (NCL[1] // 128) * 256]  # class col offsets in W
    ICOLS = 8 + NSUB * icols_bin
    NCALL = NSUB * ncalls_bin

    nc = bacc.Bacc("TRN2", num_swdge_queues=4)
    shard = nc.declare_dram_parameter(
        "shard", [SHARD, VEC], mybir.dt.float16, isOutput=False
    )
    idx = nc.declare_dram_parameter("idx", [P, ICOLS], mybir.dt.int16, isOutput=False)
    cnt = nc.declare_dram_parameter("cnt", [1, NCALL], mybir.dt.int32, isOutput=False)
    out = nc.declare_dram_parameter(
        "out", [P, NSUB * W], mybir.dt.float16, isOutput=True
    )

    sem_in = nc.alloc_semaphore("sem_in")
    sem_warm = nc.alloc_semaphore("sem_warm")
    # per-bin gather sems; bin 3's pair calls get their own so the tiny
    # pair region is the only post-gather write.
    sem_g = [nc.alloc_semaphore(f"sem_g{s}") for s in range(NSUB + 1)]
    sem_out = nc.alloc_semaphore()

    idx_sb = nc.alloc_sbuf_tensor("idx_sb", [P, ICOLS], mybir.dt.int16).ap()
    cnt_sb = nc.alloc_sbuf_tensor("cnt_sb", [1, NCALL], mybir.dt.int32).ap()
    warm_out = nc.alloc_sbuf_tensor("warm_out", [P, 1, VEC], mybir.dt.float16).ap()
    g_bufs = [
        nc.alloc_sbuf_tensor(f"g{s}", [P, W], mybir.dt.float16).ap()
        for s in range(NSUB)
    ]

    nc.gpsimd.load_library(mlp)
    # cnt + warm idx cols first (threshold 32), then one idx DMA per bin
    # on the same HWDGE ring: FIFO completion => sem_in thresholds.
    nc.sync.dma_start(out=cnt_sb[:], in_=cnt[:, :]).then_inc(sem_in, 16)
    nc.sync.dma_start(out=idx_sb[:, 0:8], in_=idx[:, 0:8]).then_inc(sem_in, 16)
    for s in range(NSUB):
        a, b = 8 + s * icols_bin, 8 + (s + 1) * icols_bin
        nc.sync.dma_start(out=idx_sb[:, a:b], in_=idx[:, a:b]).then_inc(sem_in, 16)

    warm_reg = nc.gpsimd.to_reg(128)
    cregs = [nc.gpsimd.alloc_register(name=f"creg{t}") for t in range(NCALL)]
    nc.gpsimd.wait_ge(sem_in, 16)
    for t in range(NCALL):
        nc.gpsimd.reg_load(cregs[t], cnt_sb[0:1, t : t + 1])

    # Warmup: one tiny gather per queue (row 0 x128) while the idx DMAs
    # are still in flight; absorbs the ~9us cold-queue SWDGE init.
    nc.gpsimd.wait_ge(sem_in, 32)
    for q in range(4):
        nc.gpsimd.dma_gather(
            warm_out[:, :, :],
            shard[0:RSUB, :],
            idx_sb[:, 0:8],
            128,
            warm_reg,
            VEC,
            queue_num=q,
        ).then_inc(sem_warm, 16)

    qn = 0
    t = 0
    for s in range(NSUB):
        nc.gpsimd.wait_ge(sem_in, 32 + 16 * (s + 1))
        views = [
            shard[s * RSUB : (s + 1) * RSUB, :],
            shard[s * RSUB : (s + 1) * RSUB, :].rearrange(
                "(a two) v -> a (two v)", two=2
            ),
            shard[s * RSUB + 1 : (s + 1) * RSUB - 1, :].rearrange(
                "(a two) v -> a (two v)", two=2
            ),
        ]
        for c in range(NCLS):
            ibase = 8 + s * icols_bin + sum(NCL[:c]) // 16
            for o, sz in chunks[c]:
                sem = sem_g[s] if (s < NSUB - 1 or c == 0) else sem_g[NSUB]
                dst = g_bufs[s][
                    :, roff[c] + (o // 128) * ELEM[c] : roff[c] + ((o + sz) // 128) * ELEM[c]
                ].rearrange("p (k e) -> p k e", e=ELEM[c])
                nc.gpsimd.dma_gather(
                    dst,
                    views[c],
                    idx_sb[:, ibase + o // 16 : ibase + (o + sz) // 16],
                    sz,
                    cregs[t],
                    ELEM[c],
                    queue_num=qn % 4,
                ).then_inc(sem, 16)
                qn += 1
                t += 1
    assert t == NCALL

    nch0 = len(chunks[0])
    npair_calls = len(chunks[1]) + len(chunks[2])
    writes = []  # (engine_idx, sem, need, col0, col1) cols within bin region
    for s in range(NSUB - 1):
        writes.append((s % 2, s, sem_g[s], 16 * ncalls_bin, 0, W))
    writes.append(((NSUB - 1) % 2, NSUB - 1, sem_g[NSUB - 1], 16 * nch0, 0, NCL[0]))
    writes.append((NSUB % 2, NSUB - 1, sem_g[NSUB], 16 * npair_calls, NCL[0], W))
    for ei, s, sem, need, c0, c1 in writes:
        eng = nc.sync if ei == 0 else nc.scalar
        eng.wait_ge(sem, need)
        eng.dma_start(
            out=out[:, s * W + c0 : s * W + c1], in_=g_bufs[s][:, c0:c1]
        ).then_inc(sem_out, 16)
    nc.sync.wait_ge(sem_out, 16 * len(writes))
    nc.sync.wait_ge(sem_warm, 16 * 4)
    nc.finalize()
    return nc


def _wrap_cols(vals: np.ndarray, N: int, ecount: int) -> np.ndarray:
    """int16 idx block [16, N//16]: element i at [i%16, i//16]; slots
    [len(vals), ecount) hold 0 (valid row, gathered then ignored), slots
    [ecount, N) hold -1 (skipped by the ucode)."""
    li = np.full(N, -1, np.int16)
    li[:ecount] = 0
    li[: len(vals)] = vals.astype(np.int16)
    return li.reshape(N // 16, 16).T


def _gather_on_device(table_f16: np.ndarray, uniq: np.ndarray) -> np.ndarray:
    """emb[i] = table[uniq[i]] (fp16) computed on 8 NeuronCores."""
    global LAST_RUN
    from concourse.bass_utils import run_bass_kernel_spmd

    total = uniq.shape[0]
    nbins = NCORES * NSUB
    bin_id = (uniq // RSUB).astype(np.int32)
    local = (uniq - bin_id.astype(np.int64) * RSUB).astype(np.int32)
    counts = np.bincount(bin_id, minlength=nbins)
    assert counts.sum() == total
    bin_start = np.concatenate(([0], np.cumsum(counts)))

    # Decompose every bin; record per-class (positions, view idxs).
    dec = []  # dec[b] = [(pos, vidx)] * NCLS
    ncls_max = [0] * NCLS
    for b in range(nbins):
        parts = _decompose(local[bin_start[b] : bin_start[b + 1]])
        dec.append(parts)
        for c in range(NCLS):
            ncls_max[c] = max(ncls_max[c], len(parts[c][0]))
    NCL = [max(P, ((m + P - 1) // P) * P) for m in ncls_max]
    chunks = [_chunks_of(NCL[c]) for c in range(NCLS)]
    ncalls_bin = sum(len(ch) for ch in chunks)
    icols_bin = sum(NCL) // 16
    W = NCL[0] + (NCL[1] // 128) * 256 + (NCL[2] // 128) * 256
    roff = [0, NCL[0], NCL[0] + (NCL[1] // 128) * 256]

    in_maps = []
    for core in range(NCORES):
        blocks = [np.zeros((16, 8), np.int16)]  # warm cols
        cvals = []
        for s in range(NSUB):
            b = core * NSUB + s
            for c in range(NCLS):
                vidx = dec[b][c][1]
                n = len(vidx)
                o_last = chunks[c][-1][0]
                ecount = max(n, o_last + 16)
                blocks.append(_wrap_cols(vidx, NCL[c], ecount))
                for o, sz in chunks[c]:
                    cvals.append(min(ecount - o, sz))
        in_maps.append(
            {
                "shard": np.ascontiguousarray(
                    table_f16[core * SHARD : (core + 1) * SHARD]
                ),
                "idx": np.ascontiguousarray(
                    np.tile(np.concatenate(blocks, axis=1), (8, 1))
                ),
                "cnt": np.array([cvals], np.int32),
            }
        )

    nc = _build_program(NCL, ncalls_bin)
    LAST_RUN = run_bass_kernel_spmd(nc, in_maps, list(range(NCORES)))
    res = LAST_RUN.results

    emb = np.empty((total, VEC), np.float16)
    for core in range(NCORES):
        o = np.asarray(res[core]["out"])
        for s in range(NSUB):
            b = core * NSUB + s
            bs = bin_start[b]
            reg = o[:, s * W : (s + 1) * W]
            for c in range(NCLS):
                pos = dec[b][c][0]
                n = len(pos)
                if n == 0:
                    continue
                seg = reg[:, roff[c] : roff[c] + (NCL[c] // 128) * ELEM[c]]
                rows = (
                    seg.reshape(P, NCL[c] // 128, ELEM[c])
                    .transpose(1, 0, 2)
                    .reshape(-1, ELEM[c])[:n]
                )
                if c == 0:
                    emb[bs + pos] = rows
                else:
                    pr = rows.reshape(n, 2, VEC)
                    emb[bs + pos] = pr[:, 0]
                    emb[bs + pos + 1] = pr[:, 1]
    return emb


def kernel(table, row_offsets, value_tensors, nnz_array=None, output_shape=None):
    table = np.asarray(table, dtype=np.float32)
    assert table.shape == (VOCAB, VEC)
    v = np.asarray(value_tensors).astype(np.int64).ravel()
    total = v.shape[0]

    table_f16 = table.astype(np.float16)
    uniq, inverse = np.unique(v, return_inverse=True)
    emb_u = _gather_on_device(table_f16, uniq)
    emb = emb_u[inverse].astype(np.float32)

    n_rows = BATCH * SLOTS
    ro = np.asarray(row_offsets).astype(np.int64).ravel()
    if total == n_rows and np.array_equal(ro, np.arange(total + 1)):
        return emb.reshape(BATCH, SLOTS, VEC)
    # General CSR fallback (never hit with the reference's arange offsets):
    # sum-combine values per segment on the host.
    seg = np.searchsorted(ro, np.arange(total), side="right") - 1
    combined = np.zeros((n_rows, VEC), np.float32)
    np.add.at(combined, seg, emb)
    return combined.reshape(BATCH, SLOTS, VEC)


# revision 6
# speedup vs baseline: 1.1055x; 1.0272x over previous
"""Embedding lookup (gather) kernel for Trainium2, 8 NeuronCores.

Problem: out[i] = table[value_tensors[i]] for 212992 indices into a
[1M, 128] f32 table, reshaped to [8192, 26, 128]. (row_offsets is
arange, so the CSR segment-sum is the identity; a host-side fallback
handles the general case.)

Sharding: model-parallel by table row (range partition). The table is
split into 32 range bins of 31250 rows; core c owns bins 4c..4c+3.
The host routes each lookup index to its owning bin, each core gathers
its rows on-device with the SWDGE dma_gather instruction, and the host
scatters the gathered rows back to the original positions (the
"all-to-all" step of HugeCTR's localized embedding, done at unshard
time).

Perf strategy. Measured path: f32 one-desc-per-row baseline 126us;
fp16 tables/outputs halve both HBM directions -> 88us; the gather is
then DESCRIPTOR-rate bound (~100 descs/us/queue at 256B or 512B;
engine-contiguous idx permutation measured neutral -> fixed per-desc
cost, not HBM page locality). So:
  - fp16 table + fp16 output, host upconverts (max rel err 2^-11 vs
    the 2e-2 gate); dedupe indices on host (~10% repeats).
  - WINDOW DESCRIPTORS: the sorted unique rows are greedily covered by
    windows of <= 4 consecutive table rows; each window is ONE
    descriptor (elem_size = span*128 over an overlapping-stride AP
    view with elem_step=128). ~37% fewer descriptors than
    one-per-row at ~30ns/desc fixed cost.
  - runtime num_idxs via batched reg_load from a per-core count table
    + negative idx padding: pad slots generate no descriptors, so
    per-bin/class padding (compile-time shapes shared across cores) is
    nearly free. Every chunk keeps >= 16 non-negative idxs so all 16
    engine rings still fire their completion-semaphore descriptor.
  - warmup dma_gather per queue overlapped with the idx load (first
    SWDGE gather on a cold queue measured ~9us of init).
  - per-bin idx loads; few semaphores; whole-bin writes alternating
    between the two HWDGE rings (Sync/Scalar) overlap the gathers.

dma_gather layout (probed on HW, incl. runtime-reg + negative-pad +
overlapping-window-view semantics via probe_reg.py / probe_win.py):
indices are int16, wrapped over 16 partitions (ordinal i reads
idx[i % 16, i // 16]) and replicated to all 8 Q7-core partition
groups; gathered ordinal i lands at dst[i % 128, i // 128]; negative
idxs at the end generate no descriptors.
"""

import math

import numpy as np

VOCAB = 1_000_000
BATCH = 8192
SLOTS = 26
VEC = 128
NCORES = 8
NSUB = 4  # sub-shards (bins) per core; int16 gather idx needs rows <= 32767
RSUB = VOCAB // (NCORES * NSUB)  # 31250 rows per bin
SHARD = RSUB * NSUB  # 125000 rows per core
P = 128
# Idxs per dma_gather: 896 -> 56 data descs + 1 sem desc per engine ring,
# safely under the 64-descriptor packet ceiling.
CH = 896
MXSPAN = 4
NCLS = MXSPAN  # class c gathers windows of span c+1 rows
ELEM = [(c + 1) * VEC for c in range(NCLS)]

LAST_RUN = None  # BassKernelResults of the most recent device run (for test.py)


def _chunks_of(N: int):
    out = []
    o = 0
    while o < N:
        out.append((o, min(CH, N - o)))
        o += CH
    return out


def _windows(rows: np.ndarray):
    """Greedy cover of sorted unique local rows by windows of <= MXSPAN
    consecutive table rows (optimal interval count).

    Returns per span class c (span = c+1):
      starts[c]: window start rows
      pos[c], w[c], off[c]: for each covered unique -> its bin-relative
      position, window ordinal within the class, and row offset.
    """
    n = len(rows)
    starts = [np.empty(0, np.int64) for _ in range(NCLS)]
    pos = [np.empty(0, np.int64) for _ in range(NCLS)]
    wloc = [np.empty(0, np.int64) for _ in range(NCLS)]
    off = [np.empty(0, np.int64) for _ in range(NCLS)]
    if n == 0:
        return starts, pos, wloc, off
    nxt = np.searchsorted(rows, rows + MXSPAN)
    si = []
    i = 0
    while i < n:
        si.append(i)
        i = nxt[i]
    si = np.asarray(si)
    ei = np.append(si[1:], n)
    span = rows[ei - 1] - rows[si] + 1  # 1..MXSPAN
    nwin = len(si)
    wid = np.repeat(np.arange(nwin), ei - si)
    offs = rows - rows[si][wid]
    allpos = np.arange(n)
    for c in range(NCLS):
        selw = span == c + 1
        starts[c] = rows[si[selw]]
        wl = np.cumsum(selw) - 1  # class-local ordinal per window
        selu = selw[wid]
        pos[c] = allpos[selu]
        wloc[c] = wl[wid[selu]]
        off[c] = offs[selu]
    return starts, pos, wloc, off


def _build_program(NCL: list, ncalls_bin: int):
    """One SPMD program for all 8 cores. NCL[c] = padded idx slots for
    class c per bin (multiples of 128, identical across cores/bins).

    Per core:
      shard [SHARD, VEC] fp16   - this core's 4 bins, concatenated
      idx   [P, ICOLS] i16      - [8 warm cols][bin0 c0..c3][bin1 ...]
      cnt   [1, NCALL] i32      - per-gather-call runtime num_idxs
      out   [P, NSUB*W] fp16    - W = per-bin output cols
    """
    import bass_rust
    import concourse.bacc as bacc
    from concourse import mybir
    from concourse.library_config import mlp

    chunks = [_chunks_of(NCL[c]) for c in range(NCLS)]
    icols_bin = sum(NCL) // 16
    ccols = [(NCL[c] // 128) * ELEM[c] for c in range(NCLS)]
    roff = [0] + list(np.cumsum(ccols))[:-1]
    W = sum(ccols)
    ICOLS = 8 + NSUB * icols_bin
    NCALL = NSUB * ncalls_bin

    nc = bacc.Bacc("TRN2", num_swdge_queues=4)
    shard = nc.declare_dram_parameter(
        "shard", [SHARD, VEC], mybir.dt.float16, isOutput=False
    )
    idx = nc.declare_dram_parameter("idx", [P, ICOLS], mybir.dt.int16, isOutput=False)
    cnt = nc.declare_dram_parameter("cnt", [1, NCALL], mybir.dt.int32, isOutput=False)
    out = nc.declare_dram_parameter(
        "out", [P, NSUB * W], mybir.dt.float16, isOutput=True
    )

    sem_in = nc.alloc_semaphore("sem_in")
    sem_warm = nc.alloc_semaphore("sem_warm")
    # per-bin gather sems; bin 3's multi-row classes get their own so
    # the post-gather write tail is only that region.
    sem_g = [nc.alloc_semaphore(f"sem_g{s}") for s in range(NSUB + 1)]
    sem_out = nc.alloc_semaphore()

    idx_sb = nc.alloc_sbuf_tensor("idx_sb", [P, ICOLS], mybir.dt.int16).ap()
    cnt_sb = nc.alloc_sbuf_tensor("cnt_sb", [1, NCALL], mybir.dt.int32).ap()
    warm_out = nc.alloc_sbuf_tensor("warm_out", [P, 1, VEC], mybir.dt.float16).ap()
    g_bufs = [
        nc.alloc_sbuf_tensor(f"g{s}", [P, W], mybir.dt.float16).ap()
        for s in range(NSUB)
    ]

    nc.gpsimd.load_library(mlp)
    # cnt + warm idx cols first (threshold 32), then one idx DMA per bin
    # on the same HWDGE ring: FIFO completion => sem_in thresholds.
    nc.sync.dma_start(out=cnt_sb[:], in_=cnt[:, :]).then_inc(sem_in, 16)
    nc.sync.dma_start(out=idx_sb[:, 0:8], in_=idx[:, 0:8]).then_inc(sem_in, 16)
    for s in range(NSUB):
        a, b = 8 + s * icols_bin, 8 + (s + 1) * icols_bin
        nc.sync.dma_start(out=idx_sb[:, a:b], in_=idx[:, a:b]).then_inc(sem_in, 16)

    warm_reg = nc.gpsimd.to_reg(128)
    cregs = [nc.gpsimd.alloc_register(name=f"creg{t}") for t in range(NCALL)]
    nc.gpsimd.wait_ge(sem_in, 16)
    nc.gpsimd.reg_load(cregs, cnt_sb[0:1, 0:NCALL])  # one batched load

    # Warmup: one tiny gather per queue (row 0 x128) while the idx DMAs
    # are still in flight; absorbs the ~9us cold-queue SWDGE init.
    nc.gpsimd.wait_ge(sem_in, 32)
    for q in range(4):
        nc.gpsimd.dma_gather(
            warm_out[:, :, :],
            shard[0:RSUB, :],
            idx_sb[:, 0:8],
            128,
            warm_reg,
            VEC,
            queue_num=q,
        ).then_inc(sem_warm, 16)

    qn = 0
    t = 0
    for s in range(NSUB):
        nc.gpsimd.wait_ge(sem_in, 32 + 16 * (s + 1))
        for c in range(NCLS):
            L = c + 1
            view = shard[s * RSUB : s * RSUB + (RSUB - L + 1), :].copy()
            view.ap = bass_rust.VecI64Pair([[VEC, RSUB - L + 1], [1, ELEM[c]]])
            ibase = 8 + s * icols_bin + sum(NCL[:c]) // 16
            for o, sz in chunks[c]:
                sem = sem_g[s] if (s < NSUB - 1 or c == 0) else sem_g[NSUB]
                dst = g_bufs[s][
                    :,
                    roff[c] + (o // 128) * ELEM[c] : roff[c]
                    + ((o + sz) // 128) * ELEM[c],
                ].rearrange("p (k e) -> p k e", e=ELEM[c])
                nc.gpsimd.dma_gather(
                    dst,
                    view,
                    idx_sb[:, ibase + o // 16 : ibase + (o + sz) // 16],
                    sz,
                    cregs[t],
                    ELEM[c],
                    elem_step=VEC,
                    queue_num=qn % 4,
                ).then_inc(sem, 16)
                qn += 1
                t += 1
    assert t == NCALL

    nch0 = len(chunks[0])
    nrest = ncalls_bin - nch0
    writes = []  # (engine_idx, bin, sem, need, col0, col1)
    for s in range(NSUB - 1):
        writes.append((s % 2, s, sem_g[s], 16 * ncalls_bin, 0, W))
    writes.append(((NSUB - 1) % 2, NSUB - 1, sem_g[NSUB - 1], 16 * nch0, 0, ccols[0]))
    writes.append((NSUB % 2, NSUB - 1, sem_g[NSUB], 16 * nrest, ccols[0], W))
    for ei, s, sem, need, c0, c1 in writes:
        eng = nc.sync if ei == 0 else nc.scalar
        eng.wait_ge(sem, need)
        eng.dma_start(
            out=out[:, s * W + c0 : s * W + c1], in_=g_bufs[s][:, c0:c1]
        ).then_inc(sem_out, 16)
    nc.sync.wait_ge(sem_out, 16 * len(writes))
    nc.sync.wait_ge(sem_warm, 16 * 4)
    nc.finalize()
    return nc


def _wrap_cols(vals: np.ndarray, N: int, ecount: int) -> np.ndarray:
    """int16 idx block [16, N//16]: element i at [i%16, i//16]; slots
    [len(vals), ecount) hold 0 (valid row, gathered then ignored), slots
    [ecount, N) hold -1 (skipped by the ucode)."""
    li = np.full(N, -1, np.int16)
    li[:ecount] = 0
    li[: len(vals)] = vals.astype(np.int16)
    return li.reshape(N // 16, 16).T


def _gather_on_device(table_f16: np.ndarray, uniq: np.ndarray) -> np.ndarray:
    """emb[i] = table[uniq[i]] (fp16) computed on 8 NeuronCores."""
    global LAST_RUN
    from concourse.bass_utils import run_bass_kernel_spmd

    total = uniq.shape[0]
    nbins = NCORES * NSUB
    bin_id = (uniq // RSUB).astype(np.int32)
    local = (uniq - bin_id.astype(np.int64) * RSUB).astype(np.int32)
    counts = np.bincount(bin_id, minlength=nbins)
    assert counts.sum() == total
    bin_start = np.concatenate(([0], np.cumsum(counts)))

    dec = []  # dec[b] = (starts, pos, w, off) per class
    ncls_max = [0] * NCLS
    for b in range(nbins):
        parts = _windows(local[bin_start[b] : bin_start[b + 1]])
        dec.append(parts)
        for c in range(NCLS):
            ncls_max[c] = max(ncls_max[c], len(parts[0][c]))
    NCL = [max(P, ((m + P - 1) // P) * P) for m in ncls_max]
    chunks = [_chunks_of(NCL[c]) for c in range(NCLS)]
    ncalls_bin = sum(len(ch) for ch in chunks)
    icols_bin = sum(NCL) // 16
    ccols = [(NCL[c] // 128) * ELEM[c] for c in range(NCLS)]
    roff = [0] + list(np.cumsum(ccols))[:-1]
    W = sum(ccols)

    in_maps = []
    for core in range(NCORES):
        blocks = [np.zeros((16, 8), np.int16)]  # warm cols
        cvals = []
        for s in range(NSUB):
            b = core * NSUB + s
            starts = dec[b][0]
            for c in range(NCLS):
                n = len(starts[c])
                o_last = chunks[c][-1][0]
                ecount = max(n, o_last + 16)
                blocks.append(_wrap_cols(starts[c], NCL[c], ecount))
                for o, sz in chunks[c]:
                    cvals.append(min(ecount - o, sz))
        in_maps.append(
            {
                "shard": np.ascontiguousarray(
                    table_f16[core * SHARD : (core + 1) * SHARD]
                ),
                "idx": np.ascontiguousarray(
                    np.tile(np.concatenate(blocks, axis=1), (8, 1))
                ),
                "cnt": np.array([cvals], np.int32),
            }
        )

    nc = _build_program(NCL, ncalls_bin)
    LAST_RUN = run_bass_kernel_spmd(nc, in_maps, list(range(NCORES)))
    res = LAST_RUN.results

    emb = np.empty((total, VEC), np.float16)
    for core in range(NCORES):
        o = np.asarray(res[core]["out"])
        for s in range(NSUB):
            b = core * NSUB + s
            bs = bin_start[b]
            _, pos, wloc, off = dec[b]
            reg = o[:, s * W : (s + 1) * W]
            for c in range(NCLS):
                if len(pos[c]) == 0:
                    continue
                nw = int(wloc[c].max()) + 1
                seg = reg[:, roff[c] : roff[c] + ccols[c]]
                wins = (
                    seg.reshape(P, NCL[c] // 128, ELEM[c])
                    .transpose(1, 0, 2)
                    .reshape(-1, ELEM[c])[:nw]
                    .reshape(nw, c + 1, VEC)
                )
                emb[bs + pos[c]] = wins[wloc[c], off[c]]
    return emb


def kernel(table, row_offsets, value_tensors, nnz_array=None, output_shape=None):
    table = np.asarray(table, dtype=np.float32)
    assert table.shape == (VOCAB, VEC)
    v = np.asarray(value_tensors).astype(np.int64).ravel()
    total = v.shape[0]

    table_f16 = table.astype(np.float16)
    uniq, inverse = np.unique(v, return_inverse=True)
    emb_u = _gather_on_device(table_f16, uniq)
    emb = emb_u[inverse].astype(np.float32)

    n_rows = BATCH * SLOTS
    ro = np.asarray(row_offsets).astype(np.int64).ravel()
    if total == n_rows and np.array_equal(ro, np.arange(total + 1)):
        return emb.reshape(BATCH, SLOTS, VEC)
    # General CSR fallback (never hit with the reference's arange offsets):
    # sum-combine values per segment on the host.
    seg = np.searchsorted(ro, np.arange(total), side="right") - 1
    combined = np.zeros((n_rows, VEC), np.float32)
    np.add.at(combined, seg, emb)
    return combined.reshape(BATCH, SLOTS, VEC)
